# revision 40
# baseline (speedup 1.0000x reference)
"""Causal self-attention (RoPE, 16 heads) on 8 TRN2 NeuronCores.

Problem: x[4,2048,2048] @ Wqkv -> RoPE(q,k) -> causal softmax(qk^T/sqrt(128)) @ v
         -> out proj Wout.  B=4, S=2048, D=2048, H=16, DH=128.

Sharding: tensor-parallel over heads. Each of the 8 cores computes 2 heads:
QKV projection columns for its heads, RoPE, attention, and its partial of the
output projection (row-sharded Wout). Host sums the 8 partials (+bout).

Design (vs the 903us v1 two-phase fp32r baseline; ~666us fast-mode):
  * bf16 operands everywhere (fp32 PSUM accumulation) — same PE rate as
    float32r, but cheaper weight loads, 2x DVE elementwise, half the DMA.
  * Fully fused per-batch pipeline: QKV+RoPE -> attention -> out-proj with
    Q^T/K^T/V/O^T resident in SBUF (no DRAM scratch round trip, no phase
    barrier, no per-head reload stalls).
  * Causal trimming: for the diagonal 128-k chunk dg, the moving q-range
    starts at dg*128 (N in {512,384,256,128}); only the leading 128-wide
    diagonal block needs the 0/1 mask (applied in place on DVE).
  * Attention latency chain (st -> exp on ACT -> av) hidden by interleaving
    both heads' chains per ki step and emitting the out-projection row-tiles
    of query tile qi right after qi completes (PE filler work); each batch's
    last out-proj group is deferred past the next batch's first QKV tile so
    the boundary always has ready PE work.
  * Softmax denominator via ones-matmul into PSUM; non-diagonal exp chunks
    are pre-summed in groups of four on DVE (bf16) and diagonal chunks in
    overlapping pairs (dg1 into dg0's q-range in place, dg3 into dg2's), so
    ~1/4 as many ones-matmuls stream through the PE.
  * reciprocal_approx_fast for 1/l (~5x faster than DVE reciprocal);
    PSUM->SBUF y copies on DVE as fp32->bf16 casts (keeps the in-order ACT
    queue free for exp); y partials in bf16, summed on host in fp64.
  * Startup: first x tile + first weight quarter interleaved in small DMAs
    ahead of all bulk loads (first matmul at ~14us instead of ~38us).
PSUM budget (8 banks): mm(2, shared QKV-acc/out-proj) + st(2) +
  av_h0/av_h1/lps_h0/lps_h1 (1 each).
"""

import math

import numpy as np


def _ensure_imports():
    try:
        import concourse.bass  # noqa: F401
    except ImportError:
        import sys
        for p in (
            "/root/.axon_site",
            "/root/.axon_site/_ro/trn_rl_repo",
            "/root/.axon_site/_ro/pypackages",
            "/opt/trn_rl_repo",
        ):
            if p not in sys.path:
                sys.path.append(p)


DH = 128
TOK = 512            # token tile (matmul moving free dim)
SHUF_MASK = [(i + 16) % 32 for i in range(32)]


def _perm_orig_of_p():
    """orig head-dim index stored at partition p, for the RoPE layout.

    Partition p = 32*quad + j. Rotation pair index i = 16*quad + (j % 16).
    j < 16 holds the even element (2i), j >= 16 holds the odd (2i+1).
    """
    orig = np.empty(DH, dtype=np.int64)
    for p in range(DH):
        quad, j = divmod(p, 32)
        i = 16 * quad + (j % 16)
        orig[p] = 2 * i if j < 16 else 2 * i + 1
    return orig


def _build_program(B, S, D, HPC):
    """Build the per-core SPMD program. Returns compiled Bacc."""
    import concourse.mybir as mybir
    import concourse.tile as tile
    from concourse import bacc
    from contextlib import ExitStack

    F32 = mybir.dt.float32
    BF16 = mybir.dt.bfloat16
    AF = mybir.ActivationFunctionType
    OP = mybir.AluOpType

    T = B * S
    NKO = D // 128           # contraction chunks for projections
    QCOLS = 2 * HPC          # q + k col-tiles of 128
    VCOLS = HPC * 128
    WCOLS = QCOLS * 128 + VCOLS
    NQI = S // TOK           # q tiles per (b,h)
    NDC = TOK // 128         # 128-chunks per token tile (diag masks)
    NDO = D // TOK           # output Dout tiles
    NKV = S // 128           # v chunks per batch
    scale = 1.0 / math.sqrt(DH)

    nc = bacc.Bacc()
    xT = nc.dram_tensor("xT", [D, T], BF16, kind="ExternalInput")
    w_c = nc.dram_tensor("w_c", [D, WCOLS], BF16, kind="ExternalInput")
    wout = nc.dram_tensor("wout", [VCOLS, D], BF16, kind="ExternalInput")
    cosP = nc.dram_tensor("cosP", [128, S], BF16, kind="ExternalInput")
    sinP = nc.dram_tensor("sinP", [128, S], BF16, kind="ExternalInput")
    maskT = nc.dram_tensor("maskT", [128, NDC, TOK], BF16, kind="ExternalInput")
    ones = nc.dram_tensor("ones", [128, 128], BF16, kind="ExternalInput")
    qb = nc.dram_tensor("qb", [128, QCOLS], F32, kind="ExternalInput")
    vb = nc.dram_tensor("vb", [128, VCOLS], F32, kind="ExternalInput")
    y = nc.dram_tensor("y", [T, D], BF16, kind="ExternalOutput")

    xTr = xT.rearrange("(ko p) t -> p ko t", p=128)
    w_r = w_c.rearrange("(ko p) c -> p ko c", p=128)
    wout_r = wout.rearrange("(h p) d -> p h d", p=128)

    with tile.TileContext(nc) as tc:
        with ExitStack() as ctx:
            s1 = ctx.enter_context(tc.tile_pool(name="singles", bufs=1))
            xp = ctx.enter_context(tc.tile_pool(name="xp", bufs=2))
            qkvp = ctx.enter_context(tc.tile_pool(name="qkvp", bufs=2))
            wk = ctx.enter_context(tc.tile_pool(name="wk", bufs=2))
            ptp = ctx.enter_context(tc.tile_pool(name="ptp", bufs=4))
            ptq = ctx.enter_context(tc.tile_pool(name="ptq", bufs=10))
            ysp = ctx.enter_context(tc.tile_pool(name="ysp", bufs=6))
            psA = ctx.enter_context(
                tc.tile_pool(name="psA", bufs=2, space="PSUM"))
            psB = ctx.enter_context(
                tc.tile_pool(name="psB", bufs=2, space="PSUM"))
            psC = ctx.enter_context(
                tc.tile_pool(name="psC", bufs=1, space="PSUM"))

            # ---- resident tensors -------------------------------------
            # Issue order matters for startup latency: the first x tile and
            # the first weight quarter go first so the opening matmul group
            # isn't queued behind bulk loads.
            xt00 = xp.tile([128, NKO, TOK], BF16, tag="xt")
            w_sb = s1.tile([128, NKO, WCOLS], BF16)
            ck = NKO // 8
            # interleave the first x tile and the weights at eighth
            # granularity so the opening 16-ko matmul group never outruns
            # the weight chunks still in flight
            for i in range(8):
                nc.sync.dma_start(out=xt00[:, i * ck:(i + 1) * ck, :],
                                  in_=xTr[:, i * ck:(i + 1) * ck, 0:TOK])
                nc.sync.dma_start(out=w_sb[:, i * ck:(i + 1) * ck, :],
                                  in_=w_r[:, i * ck:(i + 1) * ck, :])
            qb_sb = s1.tile([128, QCOLS], F32)
            vb_sb = s1.tile([128, VCOLS], F32)
            nc.sync.dma_start(out=qb_sb, in_=qb[:, :])
            nc.sync.dma_start(out=vb_sb, in_=vb[:, :])
            cos_sb = s1.tile([128, S], BF16)
            sin_sb = s1.tile([128, S], BF16)
            nc.sync.dma_start(out=cos_sb, in_=cosP[:, :])
            nc.sync.dma_start(out=sin_sb, in_=sinP[:, :])
            # allocated now, DMA'd after the first xt tile (see loop)
            wout_sb = s1.tile([128, HPC, D], BF16)
            mask_sb = s1.tile([128, NDC, TOK], BF16)
            ones_sb = s1.tile([128, 128], BF16)

            def emit_c(b0_, ot_, qi_, use_act=False):
                # out-projection row-tiles for query tile qi_ of batch at b0_
                for qs in range(qi_ * NDC, (qi_ + 1) * NDC):
                    ysb = ysp.tile([128, D], BF16, tag="ysb",
                                   name=f"ysb_{b0_}_{qs}")
                    for do in range(NDO):
                        yp = psA.tile([128, TOK], F32, tag="mm",
                                      name=f"yp_{b0_}_{qs}_{do}")
                        for h in range(HPC):
                            nc.tensor.matmul(
                                yp, ot_[:, h, qs * 128:(qs + 1) * 128],
                                wout_sb[:, h, do * TOK:(do + 1) * TOK],
                                start=(h == 0), stop=(h == HPC - 1))
                        if use_act:
                            # batch-boundary group: ACT is idle here and the
                            # DVE queue is backed up behind the recip chain —
                            # drain the mm slots fast so the next batch's
                            # QKV accumulations aren't blocked
                            nc.scalar.activation(
                                ysb[:, do * TOK:(do + 1) * TOK], yp, AF.Copy)
                        else:
                            nc.vector.tensor_copy(
                                ysb[:, do * TOK:(do + 1) * TOK], yp)
                    nc.sync.dma_start(
                        out=y[b0_ + qs * 128:b0_ + (qs + 1) * 128, :],
                        in_=ysb)

            pending_c = None
            for b in range(B):
                b0 = b * S
                qt = qkvp.tile([128, HPC, S], BF16, tag="qt")
                kt = qkvp.tile([128, HPC, S], BF16, tag="kt")
                vt = qkvp.tile([128, NKV, VCOLS], BF16, tag="vt")
                ot = qkvp.tile([128, HPC, S], BF16, tag="ot")
                for t in range(NQI):
                    # ---- A(t): QKV projection + RoPE for token tile t ----
                    tg = b0 + t * TOK
                    if b == 0 and t == 0:
                        xt = xt00  # prefetched before the resident loads
                        # bulk B/C-phase inputs: issued behind the first xt
                        nc.sync.dma_start(out=mask_sb,
                                          in_=maskT.rearrange("p n s -> p n s"))
                        nc.sync.dma_start(out=ones_sb, in_=ones[:, :])
                        for h in range(HPC):
                            nc.sync.dma_start(out=wout_sb[:, h, :],
                                              in_=wout_r[:, h, :])
                    else:
                        xt = xp.tile([128, NKO, TOK], BF16, tag="xt")
                        for i in range(2):
                            hk = NKO // 2
                            nc.sync.dma_start(
                                out=xt[:, i * hk:(i + 1) * hk, :],
                                in_=xTr[:, i * hk:(i + 1) * hk, tg:tg + TOK])
                    for c4 in range(QCOLS):
                        acc = psA.tile([128, TOK], F32, tag="mm")
                        for ko in range(NKO):
                            nc.tensor.matmul(
                                acc, w_sb[:, ko, c4 * 128:(c4 + 1) * 128],
                                xt[:, ko, :],
                                start=(ko == 0), stop=(ko == NKO - 1))
                        raw = wk.tile([128, TOK], BF16, tag="raw")
                        nc.scalar.activation(raw, acc, AF.Identity,
                                             bias=qb_sb[:, c4:c4 + 1])
                        sw = wk.tile([128, TOK], BF16, tag="sw")
                        # partition-only permute: bitcast to u32 halves the
                        # streamed element count (pairs along free dim)
                        nc.vector.stream_shuffle(
                            sw.bitcast(mybir.dt.uint32),
                            raw.bitcast(mybir.dt.uint32), SHUF_MASK)
                        m1 = wk.tile([128, TOK], BF16, tag="m1")
                        nc.vector.tensor_tensor(
                            m1, raw, cos_sb[:, t * TOK:(t + 1) * TOK],
                            op=OP.mult)
                        m2 = wk.tile([128, TOK], BF16, tag="m2")
                        nc.vector.tensor_tensor(
                            m2, sw, sin_sb[:, t * TOK:(t + 1) * TOK],
                            op=OP.mult)
                        dst = qt if c4 < HPC else kt
                        nc.vector.tensor_tensor(
                            dst[:, c4 % HPC, t * TOK:(t + 1) * TOK], m1, m2,
                            op=OP.add)
                    for sub in range(NDC):
                        accv = psA.tile([128, VCOLS], F32, tag="mm")
                        for ko in range(NKO):
                            nc.tensor.matmul(
                                accv, xt[:, ko, sub * 128:(sub + 1) * 128],
                                w_sb[:, ko, QCOLS * 128:WCOLS],
                                start=(ko == 0), stop=(ko == NKO - 1))
                        nc.vector.tensor_tensor(
                            vt[:, t * NDC + sub, :], accv, vb_sb, op=OP.add)

                    if t == 0 and pending_c is not None:
                        # previous batch's deferred last out-projection:
                        # emitted after this batch's first QKV tile so the
                        # batch boundary has ready PE work on both sides
                        emit_c(*pending_c)
                        pending_c = None

                # ---- B: attention per query tile (heads interleaved),
                # ---- each followed by its out-projection row-tiles (C)
                for qi in range(NQI):
                    q0 = qi * TOK
                    nki = NDC * qi + NDC
                    avs, lpss = [], []
                    for h in range(HPC):
                        av_h = psC.tile([128, TOK], F32, tag=f"av{h}",
                                        name=f"av{h}_{b}_{qi}")
                        lps_h = psC.tile([128, TOK], F32, tag=f"lps{h}",
                                         name=f"lps{h}_{b}_{qi}")
                        avs.append(av_h)
                        lpss.append(lps_h)
                    pend = [[], []]          # ungrouped non-diag pt, per head
                    pend_d = [None, None]    # unpaired diag pt, per head
                    lps_open = [False] * HPC
                    for ki in range(nki):
                        dg = ki - NDC * qi
                        qoff = max(dg, 0) * 128
                        N = TOK - qoff
                        pts = []
                        for h in range(HPC):
                            # both heads' score matmuls + exps issued first so
                            # ACT gets the pair ASAP and each head's PV work
                            # overlaps the other head's exp
                            st = psB.tile([128, TOK], F32, tag="st")
                            nc.tensor.matmul(
                                st[:, :N], kt[:, h, ki * 128:(ki + 1) * 128],
                                qt[:, h, q0 + qoff:q0 + TOK],
                                start=True, stop=True)
                            pt = ptq.tile([128, TOK], BF16, tag="pt",
                                          name=f"pt_{b}_{qi}_{ki}_{h}")
                            nc.scalar.activation(pt[:, :N], st[:, :N], AF.Exp,
                                                 scale=scale)
                            if dg >= 0:
                                # after trimming, only the leading 128-wide
                                # diagonal block is partially masked; zero it
                                # in place and feed pt to the PV matmuls
                                nc.vector.tensor_tensor(
                                    pt[:, 0:128], pt[:, 0:128],
                                    mask_sb[:, dg, qoff:qoff + 128],
                                    op=OP.mult)
                            pts.append(pt)
                        for h in range(HPC):
                            pt = pts[h]
                            nc.tensor.matmul(
                                avs[h][:, qoff:TOK],
                                vt[:, ki, h * 128:(h + 1) * 128], pt[:, :N],
                                start=(ki == 0), stop=(ki == nki - 1))
                            # softmax denominator: pre-sum groups of four
                            # non-diag exp chunks on DVE (bf16) so only one
                            # ones-matmul streams per quad (non-diag count
                            # per qi is 4*qi — always a multiple of 4);
                            # diag chunks pair (dg0+dg1, dg2+dg3) by adding
                            # the later chunk into the earlier one's
                            # overlapping q-range in place
                            if dg < 0:
                                pend[h].append(pt)
                                if len(pend[h]) < 4:
                                    continue
                                p0, p1, p2, p3 = pend[h]
                                pend[h] = []
                                pa = ptp.tile([128, TOK], BF16, tag="ppa",
                                              name=f"pa_{b}_{qi}_{ki}_{h}")
                                nc.vector.tensor_tensor(pa, p0, p1, op=OP.add)
                                pb = ptp.tile([128, TOK], BF16, tag="ppb",
                                              name=f"pb_{b}_{qi}_{ki}_{h}")
                                nc.vector.tensor_tensor(pb, p2, p3, op=OP.add)
                                pp = ptp.tile([128, TOK], BF16, tag="pp",
                                              name=f"pp_{b}_{qi}_{ki}_{h}")
                                nc.vector.tensor_tensor(pp, pa, pb, op=OP.add)
                                nc.tensor.matmul(
                                    lpss[h][:, qoff:TOK], ones_sb, pp[:, :N],
                                    start=(not lps_open[h]), stop=False)
                                lps_open[h] = True
                            elif dg in (0, 2):
                                pend_d[h] = pt
                            else:
                                base = pend_d[h]
                                pend_d[h] = None
                                # base covers q-local [qoff-128, TOK); this
                                # chunk covers [qoff, TOK) = base cols 128:
                                nc.vector.tensor_tensor(
                                    base[:, 128:128 + N], base[:, 128:128 + N],
                                    pt[:, :N], op=OP.add)
                                nc.tensor.matmul(
                                    lpss[h][:, qoff - 128:TOK], ones_sb,
                                    base[:, :N + 128],
                                    start=(not lps_open[h]),
                                    stop=(ki == nki - 1))
                                lps_open[h] = True
                    for h in range(HPC):
                        recl = wk.tile([128, TOK], F32, tag="recl")
                        nc.vector.reciprocal_approx_fast(recl, lpss[h])
                        nc.vector.tensor_tensor(
                            ot[:, h, q0:q0 + TOK], avs[h], recl, op=OP.mult)
                    if qi < NQI - 1 or b == B - 1:
                        emit_c(b0, ot, qi)
                    else:
                        pending_c = (b0, ot, qi)

            if pending_c is not None:
                emit_c(*pending_c)

    nc.compile()
    return nc


def _host_prep(x, rope_cos, rope_sin, Wqkv, bqkv, Wout, B, S, D, H, n_cores):
    """Build per-core input maps (bf16 data, fp32 biases)."""
    import ml_dtypes
    BF = ml_dtypes.bfloat16

    T = B * S
    HPC = H // n_cores
    orig = _perm_orig_of_p()
    quad_j = np.arange(DH)
    jmod = quad_j % 32
    i_of_p = (quad_j // 32) * 16 + (jmod % 16)
    sign = np.where(jmod < 16, -1.0, 1.0).astype(np.float32)

    xT = np.ascontiguousarray(x.reshape(T, D).T.astype(BF))  # [D, T]
    cosP = np.ascontiguousarray(rope_cos[:, i_of_p].T.astype(BF))
    sinP = np.ascontiguousarray((rope_sin[:, i_of_p] * sign).T.astype(BF))

    NDC = TOK // 128
    pl = np.arange(128)[:, None]
    ql = np.arange(TOK)[None, :]
    maskT = np.stack([(d * 128 + pl <= ql) for d in range(NDC)], axis=1)
    maskT = np.ascontiguousarray(maskT.astype(BF))  # [128, NDC, TOK]

    ones = np.ones((128, 128), dtype=BF)

    in_maps = []
    for c in range(n_cores):
        heads = [c * HPC + i for i in range(HPC)]
        wq = [Wqkv[:, h * DH + orig] for h in heads]
        wk = [Wqkv[:, H * DH + h * DH + orig] for h in heads]
        wv = [Wqkv[:, 2 * H * DH + h * DH:2 * H * DH + (h + 1) * DH]
              for h in heads]
        w_c = np.ascontiguousarray(
            np.concatenate(wq + wk + wv, axis=1).astype(BF))
        wout_c = np.ascontiguousarray(
            Wout[c * HPC * DH:(c + 1) * HPC * DH, :].astype(BF))
        qb_cols = ([bqkv[h * DH + orig] for h in heads] +
                   [bqkv[H * DH + h * DH + orig] for h in heads])
        qb = np.ascontiguousarray(np.stack(qb_cols, axis=1).astype(np.float32))
        vb_flat = np.concatenate(
            [bqkv[2 * H * DH + h * DH:2 * H * DH + (h + 1) * DH]
             for h in heads])
        vb = np.ascontiguousarray(
            np.broadcast_to(vb_flat[None, :], (128, HPC * DH)).astype(
                np.float32))
        in_maps.append({
            "xT": xT, "w_c": w_c, "wout": wout_c, "cosP": cosP, "sinP": sinP,
            "maskT": maskT, "ones": ones, "qb": qb, "vb": vb,
        })
    return in_maps


def _run(x, rope_cos, rope_sin, Wqkv, bqkv, Wout, bout,
         B, S, D, H, n_cores, trace=False):
    _ensure_imports()
    from concourse.bass_utils import run_bass_kernel_spmd

    HPC = H // n_cores
    import time as _time
    _t0 = _time.time()
    nc = _build_program(B, S, D, HPC)
    print(f"[kernel] build+compile wall: {_time.time() - _t0:.1f}s", flush=True)
    in_maps = _host_prep(np.asarray(x, dtype=np.float32),
                         np.asarray(rope_cos, dtype=np.float32),
                         np.asarray(rope_sin, dtype=np.float32),
                         np.asarray(Wqkv, dtype=np.float32),
                         np.asarray(bqkv, dtype=np.float32),
                         np.asarray(Wout, dtype=np.float32),
                         B, S, D, H, n_cores)
    _t0 = _time.time()
    res = run_bass_kernel_spmd(nc, in_maps, list(range(n_cores)), trace=trace)
    print(f"[kernel] spmd run wall: {_time.time() - _t0:.1f}s", flush=True)
    y = res.results[0]["y"].astype(np.float64)
    for i in range(1, n_cores):
        y += res.results[i]["y"]
    y += np.asarray(bout, dtype=np.float64)[None, :]
    out = y.astype(np.float32).reshape(B, S, D)
    return out, res


def kernel(x, rope_cos, rope_sin, Wqkv, bqkv, Wout, bout):
    out, _ = _run(x, rope_cos, rope_sin, Wqkv, bqkv, Wout, bout,
                  B=4, S=2048, D=2048, H=16, n_cores=8)
    return out
